# revision 73
# baseline (speedup 1.0000x reference)
"""Trainium2 Bass kernel for a Transformer-XL style BertLayer (relative attention).

Sharding (8 NeuronCores, full inputs in / full output out):
  Dispatch 1 (token-sharded): transposed projections quT/qvT/kT/vT/pT in
  fp8-e4m3 (weights pre-scaled x32 on host, un-scaled in the PSUM->SBUF
  activation; pos biases folded into the q projections).
  Host: reassemble; query-split for dispatch 2 (core c: batch c//4,
    queries [512*(c%4), +512)).
  Dispatch 2: attention with keys-on-partitions ([j, q] score layout).
  Rel-shift: the dense BD rect (q on partitions, position window on free)
  is written bf16 to a sheared DRAM frame (row q at column offset -q) and
  read back through the DMA XBAR transpose, landing BD^T tiles [j, q]
  directly in SBUF. Those accumulate into the content-score PSUM via
  identity matmuls; softmax denominators ride as a ones-column in V.
  Attention matmuls run fp8 (ctx in DoubleRow perf mode). Then Wo (fp8
  DoubleRow) + residual/LN1 + FFN in bf16 (exact GELU) + residual/LN2.
"""

import os
import sys
import numpy as np
import ml_dtypes

sys.path.insert(0, "/opt/trn_rl_repo")

import concourse.bass as bass
import concourse.mybir as mybir
import concourse.tile as tile
from concourse import bacc
from concourse.bass_utils import run_bass_kernel_spmd
from concourse.masks import make_identity

BF = ml_dtypes.bfloat16
E4 = ml_dtypes.float8_e4m3
F32, BF16, F32R = mybir.dt.float32, mybir.dt.bfloat16, mybir.dt.float32r
FP8 = mybir.dt.float8e4
DRPM = mybir.MatmulPerfMode.DoubleRow
AFT = mybir.ActivationFunctionType
ALU = mybir.AluOpType
AXX = mybir.AxisListType.X

B, T, H, NH, DK = 2, 2048, 768, 12, 64
P = 128
FC = H // P            # 6 feature chunks
GC = 3072 // P         # 24 intermediate chunks
Q = 512                # queries per core
NCORE = 8
WWIN = 2560            # pT window width per core
RECTW = 2176           # valid rect row width per query tile
WS = 32.0              # fp8 weight pre-scale
VBW = 80               # padded vb row width (DoubleRow needs 16-mult stride)
LN_EPS = 1e-5

_cache = {}
PROFILE = {}


# ---------------------------------------------------------------------------
# Dispatch 1: projections, token-sharded, fp8 in / fp8 out.
def _build_d1(bias_free: frozenset = frozenset()):
    nc = bacc.Bacc(None, target_bir_lowering=False)
    xT = nc.dram_tensor("xT", [P, FC, Q], FP8, kind="ExternalInput")
    posT = nc.dram_tensor("posT", [P, FC, Q], FP8, kind="ExternalInput")
    ws = {n: nc.dram_tensor(n, [P, FC, H], FP8, kind="ExternalInput")
          for n in ("Wq", "Wk", "Wv", "Wp")}
    bs = {n: nc.dram_tensor(n, [P, FC], F32, kind="ExternalInput")
          for n in ("bqu", "bqv", "bk", "bv")}
    outs = {n: nc.dram_tensor(n, [P, FC, Q], FP8, kind="ExternalOutput")
            for n in ("quT", "qvT", "kT", "vT", "pT")}

    with tile.TileContext(nc) as tc:
        with tc.tile_pool(name="sb", bufs=2) as sb, \
             tc.tile_pool(name="wp", bufs=2) as wp, \
             tc.tile_pool(name="ps", bufs=4, space="PSUM") as psp:
            xT_sb = sb.tile([P, FC, Q], FP8, tag="x")
            nc.sync.dma_start(xT_sb[:], xT[:])
            posT_sb = sb.tile([P, FC, Q], FP8, tag="p")
            nc.sync.dma_start(posT_sb[:], posT[:])
            bias_sb = {}
            for n in bs:
                t = sb.tile([P, FC], F32, tag=n)
                nc.sync.dma_start(t[:], bs[n][:])
                bias_sb[n] = t
            zcol = sb.tile([P, 1], F32, tag="z")
            nc.any.memset(zcol[:], 0.0)

            # (weight, src, [(out, bias, engine), ...])
            groups = (
                ("Wq", xT_sb, (("quT", "bqu", "act"), ("qvT", "bqv", "act"))),
                ("Wk", xT_sb, (("kT", "bk", "dve"),)),
                ("Wv", xT_sb, (("vT", "bv", "dve"),)),
                ("Wp", posT_sb, (("pT", None, "act"),)),
            )
            for wn, src, emits in groups:
                w_sb = wp.tile([P, FC, H], FP8, tag="w")
                for wc in range(3):
                    nc.sync.dma_start(w_sb[:, :, wc * 256:(wc + 1) * 256],
                                      ws[wn][:, :, wc * 256:(wc + 1) * 256])
                o_sbs = {on: sb.tile([P, FC, Q], FP8, tag=on, name=f"o_{on}")
                         for on, _, _ in emits}
                if wn in bias_free:
                    # biases all zero: pair dc chunks, halve emission count
                    for dp in range(3):
                        ps = psp.tile([P, 2, Q], F32, tag="psw", bufs=2)
                        for dh in range(2):
                            dc = 2 * dp + dh
                            for fp in range(3):
                                for qc in range(2):
                                    nc.tensor.matmul(
                                        ps[:, dh, qc * 256:(qc + 1) * 256],
                                        w_sb[:, 2 * fp:2 * fp + 2,
                                             dc * P:(dc + 1) * P],
                                        src[:, 2 * fp:2 * fp + 2,
                                            qc * 256:(qc + 1) * 256],
                                        start=(fp == 0), stop=(fp == 2),
                                        perf_mode=DRPM)
                        for oi, (on, bn, eng) in enumerate(emits):
                            if (dp + oi) % 2 == 0:
                                nc.scalar.activation(
                                    o_sbs[on][:, 2 * dp:2 * dp + 2], ps[:],
                                    AFT.Identity, bias=zcol[:, 0:1],
                                    scale=1.0 / WS)
                            else:
                                nc.vector.tensor_scalar(
                                    o_sbs[on][:, 2 * dp:2 * dp + 2], ps[:],
                                    1.0 / WS, None, op0=ALU.mult)
                    continue
                for dc in range(FC):
                    ps = psp.tile([P, Q], F32, tag="ps")
                    for fp in range(3):
                        for qc in range(2):
                            nc.tensor.matmul(
                                ps[:, qc * 256:(qc + 1) * 256],
                                w_sb[:, 2 * fp:2 * fp + 2, dc * P:(dc + 1) * P],
                                src[:, 2 * fp:2 * fp + 2,
                                    qc * 256:(qc + 1) * 256],
                                start=(fp == 0), stop=(fp == 2),
                                perf_mode=DRPM)
                    for oi, (on, bn, eng) in enumerate(emits):
                        bcol = bias_sb[bn][:, dc:dc + 1] if bn else zcol[:, 0:1]
                        if (dc + oi) % 2 == 0:
                            nc.scalar.activation(o_sbs[on][:, dc], ps[:],
                                                 AFT.Identity, bias=bcol,
                                                 scale=1.0 / WS)
                        else:
                            nc.vector.tensor_scalar(o_sbs[on][:, dc], ps[:],
                                                    bcol, 1.0 / WS,
                                                    op0=ALU.add, op1=ALU.mult)
                for on, _, _ in emits:
                    nc.gpsimd.dma_start(outs[on][:], o_sbs[on][:])
    nc.compile()
    return nc


# ---------------------------------------------------------------------------
# Dispatch 2 fast path (no attention mask).
def _build_d2(affine: bool):
    nc = bacc.Bacc(None, target_bir_lowering=False)
    kT = nc.dram_tensor("kT", [P, FC, T], FP8, kind="ExternalInput")
    quT = nc.dram_tensor("quT", [P, FC, Q], FP8, kind="ExternalInput")
    qvT = nc.dram_tensor("qvT", [P, FC, Q], FP8, kind="ExternalInput")
    pTw = nc.dram_tensor("pTw", [P, FC, WWIN], FP8, kind="ExternalInput")
    vb = nc.dram_tensor("vb", [NH, P, 16, VBW], FP8, kind="ExternalInput")
    xq = nc.dram_tensor("xq", [P, 4, H], F32, kind="ExternalInput")
    Wo = nc.dram_tensor("Wo", [P, FC, H], FP8, kind="ExternalInput")
    W1 = nc.dram_tensor("W1", [P, FC, 3072], BF16, kind="ExternalInput")
    W2 = nc.dram_tensor("W2", [P, GC, H], BF16, kind="ExternalInput")
    b1c = nc.dram_tensor("b1c", [P, GC], F32, kind="ExternalInput")
    if affine:
        # rows: 0=bo 1=b2 2=ln1_g 3=ln1_b 4=ln2_g 5=ln2_b
        vecs = nc.dram_tensor("vecs", [P, 6, H], F32, kind="ExternalInput")
    out = nc.dram_tensor("out", [P, 4, H], F32, kind="ExternalOutput")

    with tile.TileContext(nc) as tc:
        with tc.tile_pool(name="res", bufs=1) as res, \
             tc.tile_pool(name="stream", bufs=3) as stream, \
             tc.tile_pool(name="work", bufs=3) as work, \
             tc.tile_pool(name="dram", bufs=4, space="DRAM") as dpool:

            # ---------------- resident loads ----------------
            pT_sb = res.tile([P, FC, WWIN], FP8, tag="pTw")
            qvT_sb = res.tile([P, FC, Q], FP8, tag="qvT")
            nc.sync.dma_start(qvT_sb[:], qvT[:])
            quT_sb = res.tile([P, FC, Q], FP8, tag="quT")
            nc.sync.dma_start(quT_sb[:], quT[:])
            kT_sb = res.tile([P, FC, T], FP8, tag="kT")
            for hc_ in range(FC):
                nc.sync.dma_start(pT_sb[:, hc_], pTw[:, hc_])

            ident_raw = res.tile([P, P], F32, tag="idraw")
            make_identity(nc, ident_raw[:])
            ident_bf = res.tile([P, P], BF16, tag="identbf")
            nc.vector.tensor_copy(out=ident_bf[:], in_=ident_raw[:])
            eps_sb = res.tile([P, 1], F32, tag="eps")
            nc.any.memset(eps_sb[:], LN_EPS)
            zcol = res.tile([P, 1], F32, tag="zcol")
            nc.any.memset(zcol[:], 0.0)
            ones_f = res.tile([1, DK], F32, tag="onesf")
            nc.any.memset(ones_f[:], 1.0 / WS)
            ones_r = res.tile([1, DK], F32R, tag="onesr")
            nc.vector.tensor_copy(out=ones_r[:], in_=ones_f[:])

            ctxT = res.tile([P, FC, Q], FP8, tag="ctxT")
            gate_sb = res.tile([1, 1], F32, tag="gate")
            nc.any.memset(gate_sb[:], 0.0)
            xq_sb = res.tile([P, 4, H], F32, tag="xq")
            Wo_sb = res.tile([P, FC, H], FP8, tag="Wo")
            b1_sb = res.tile([P, GC], F32, tag="b1c")

            # ---------------- attention ----------------
            with tc.tile_pool(name="ps_bd", bufs=2, space="PSUM") as ps_bd, \
                 tc.tile_pool(name="ps_st", bufs=2, space="PSUM") as ps_st, \
                 tc.tile_pool(name="ps_ctx", bufs=1, space="PSUM") as ps_ctx, \
                 tc.tile_pool(name="ps_bc", bufs=1, space="PSUM") as ps_bc, \
                 tc.tile_pool(name="apool", bufs=3) as apool, \
                 tc.tile_pool(name="bpool", bufs=8) as bpool, \
                 tc.tile_pool(name="epool", bufs=4) as epool:

                CH = ((0, 512), (512, 512), (1024, 512), (1536, 512),
                      (2048, 128))

                def a_part(h, qt, state):
                    # one query-tile of the BD rect; qt-pairs share one
                    # sheared frame write. Last tile issues XBARs + V prefetch
                    hp, hc = DK * (h % 2), h // 2
                    if qt == 0:
                        state["frame"] = dpool.tile([Q, WWIN], BF16,
                                                    tag="frame", name="frame")
                    frame = state["frame"]
                    if qt % 2 == 0:
                        state["bd"] = apool.tile([P, 2, RECTW], BF16,
                                                 tag="bd", name="bd_sb")
                    bd_sb = state["bd"]
                    loc = 384 - 128 * qt
                    ev = 0
                    for off, w in CH:
                        ps = ps_bd.tile([P, 512], F32, tag="bdp")
                        nc.tensor.matmul(
                            ps[:, :w],
                            qvT_sb[hp:hp + DK, hc, qt * P:(qt + 1) * P],
                            pT_sb[hp:hp + DK, hc, loc + off:loc + off + w],
                            start=True, stop=True)
                        if ev % 2 == 0:
                            nc.vector.tensor_copy(
                                out=bd_sb[:, qt % 2, off:off + w],
                                in_=ps[:, :w])
                        else:
                            nc.scalar.activation(
                                bd_sb[:, qt % 2, off:off + w],
                                ps[:, :w], AFT.Copy)
                        ev += 1
                    if qt % 2 == 1:
                        q0 = P * (qt - 1)
                        dst = bass.AP(
                            frame.tensor,
                            frame.offset + q0 * WWIN + (384 - q0),
                            [[WWIN, P], [P * WWIN - P, 2], [1, RECTW]])
                        nc.gpsimd.dma_start(dst, bd_sb[:])
                    if qt == 3:
                        bts = []
                        for g in range(4):
                            bt = bpool.tile([P, 4, Q], BF16, tag="bt",
                                            name=f"bt{g}")
                            src = bass.AP(frame.tensor,
                                          frame.offset + 511 + 512 * g,
                                          [[WWIN - 1, Q], [1, 512]])
                            nc.sync.dma_start(bt[:], src, transpose=True)
                            bts.append(bt)
                        state["bts"] = bts
                        vb_sb = stream.tile([P, 16, VBW], FP8, tag="vb",
                                            bufs=4)
                        nc.gpsimd.dma_start(vb_sb[:], vb[h])
                        state["vb"] = vb_sb

                def b_part(h, part, state):
                    # two score j-pairs (part 0-3), then ctx norm (part 3)
                    hp, hc = DK * (h % 2), h // 2
                    bts, vb_sb = state["bts"], state["vb"]
                    if part == 0:
                        state["ctx"] = ps_ctx.tile([DK + 1, Q], F32,
                                                   tag="ctx", name="ctx")
                    e_sb = epool.tile([P, 4, Q], FP8, tag="e", name="e4")
                    ctx = state["ctx"]
                    for jp in (2 * part, 2 * part + 1):
                        st = ps_st.tile([P, 1024], F32, tag="st")
                        for half in range(2):
                            jt = 2 * jp + half
                            nc.tensor.matmul(
                                st[:, half * 512:half * 512 + 512],
                                ident_bf[:], bts[jt // 4][:, jt % 4, :],
                                start=True, stop=False)
                        for half in range(2):
                            jt = 2 * jp + half
                            nc.tensor.matmul(
                                st[:, half * 512:half * 512 + 512],
                                kT_sb[hp:hp + DK, hc, jt * P:(jt + 1) * P],
                                quT_sb[hp:hp + DK, hc, :],
                                start=False, stop=True)
                        jl = jp - 2 * part
                        nc.scalar.activation(e_sb[:, 2 * jl:2 * jl + 2, :],
                                             st[:], AFT.Exp, scale=0.125)
                        for qc in range(2):
                            nc.tensor.matmul(
                                ctx[:, qc * 256:(qc + 1) * 256],
                                vb_sb[:, 2 * jp:2 * jp + 2, :DK + 1],
                                e_sb[:, 2 * jl:2 * jl + 2,
                                     qc * 256:(qc + 1) * 256],
                                start=(jp == 0), stop=(jp == 7),
                                perf_mode=DRPM)
                    if part == 3:
                        # normalize: ctxT = ctx * (WS / den)
                        den_r = work.tile([1, Q], F32R, tag="den")
                        nc.vector.tensor_copy(out=den_r[:],
                                              in_=ctx[DK:DK + 1, :])
                        bc = ps_bc.tile([DK, Q], F32, tag="bc")
                        nc.tensor.matmul(bc[:], ones_r[:], den_r[:],
                                         start=True, stop=True)
                        bc_sb = work.tile([DK, Q], F32, tag="bc_sb")
                        nc.vector.reciprocal(bc_sb[:], bc[:])
                        nc.vector.tensor_tensor(ctxT[hp:hp + DK, hc, :],
                                                ctx[:DK, :], bc_sb[:],
                                                ALU.mult)

                states = {}
                SCHED = ((0, None), (1, None), (None, 0), (2, None),
                         (None, 1), (3, None), (None, 2), (None, 3))
                for h in range(NH + 2):
                    if h < FC:
                        nc.sync.dma_start(kT_sb[:, h], kT[:, h])
                    if h == 3:
                        # mid-attention prefetch of the post-attention
                        # weights. No-dep DMAs get hoisted to t=0 by the
                        # scheduler and clog startup; gate each with a dummy
                        # reader chained on head 0's normalized output.
                        nc.vector.tensor_tensor(gate_sb[:],
                                                ctxT[0:1, 0, 0:1],
                                                ctxT[0:1, 0, 0:1], ALU.add)
                        for tgt in (xq_sb[0:1, 0, 0:1], Wo_sb[0:1, 0, 0:1],
                                    b1_sb[0:1, 0:1]):
                            nc.vector.tensor_tensor(gate_sb[:], tgt,
                                                    gate_sb[:], ALU.add)
                        nc.sync.dma_start(xq_sb[:], xq[:])
                        nc.sync.dma_start(Wo_sb[:], Wo[:])
                        nc.sync.dma_start(b1_sb[:], b1c[:])
                    for astep, bstep in SCHED:
                        if astep is not None and h < NH:
                            if astep == 0:
                                states[h] = {}
                            a_part(h, astep, states[h])
                        if bstep is not None and h >= 2:
                            b_part(h - 2, bstep, states[h - 2])
                    if h >= 2:
                        del states[h - 2]

            # ---------------- Wo + LN1 + FFN + LN2 ----------------
            if affine:
                vecs_sb = res.tile([P, 6, H], F32, tag="vecs")
                nc.sync.dma_start(vecs_sb[:], vecs[:])

            x1f = res.tile([P, 4, H], F32, tag="x1f")
            x1T = res.tile([P, FC, Q], BF16, tag="x1T")

            def layer_norm(dst, src, g_row, b_row, bf_copy=None,
                           veng=None):
                # dst = LN(src) [* g + b]; src is an f32 (P, H) SBUF AP
                veng = veng or nc.vector
                stats = work.tile([P, 2, 6], F32, tag="stats")
                nc.vector.bn_stats(stats[:, 0], src[:, :384])
                nc.vector.bn_stats(stats[:, 1], src[:, 384:])
                mv = work.tile([P, 2], F32, tag="mv")
                nc.vector.bn_aggr(mv[:], stats[:])
                vr = work.tile([P, 1], F32, tag="vr")
                nc.vector.tensor_scalar(vr[:], mv[:, 1:2], eps_sb[:, 0:1],
                                        None, op0=ALU.add)
                rv = work.tile([P, 1], F32, tag="rv")
                nc.vector.reciprocal(rv[:], vr[:])
                rstd = work.tile([P, 1], F32, tag="rstd")
                nc.scalar.activation(rstd[:], rv[:], AFT.Sqrt)
                if affine:
                    sq = work.tile([P, H], F32, tag="sq")
                    veng.tensor_scalar(sq[:], src, mv[:, 0:1], rstd[:],
                                       op0=ALU.subtract, op1=ALU.mult)
                    veng.tensor_tensor(sq[:], sq[:], g_row, ALU.mult)
                    veng.tensor_tensor(dst, sq[:], b_row, ALU.add)
                else:
                    veng.tensor_scalar(dst, src, mv[:, 0:1], rstd[:],
                                       op0=ALU.subtract, op1=ALU.mult)
                if bf_copy is not None:
                    veng.tensor_copy(out=bf_copy, in_=dst)

            with tc.tile_pool(name="ps_ao", bufs=3, space="PSUM") as ps_ao, \
                 tc.tile_pool(name="ps_h1", bufs=2, space="PSUM") as ps_h1:
                for qt in range(4):
                    ao = ps_ao.tile([P, H], F32, tag="ao")
                    for c in range(3):
                        for n in range(3):
                            nc.tensor.matmul(
                                ao[:, n * 256:(n + 1) * 256],
                                ctxT[:, 2 * c:2 * c + 2,
                                     qt * P:(qt + 1) * P],
                                Wo_sb[:, 2 * c:2 * c + 2,
                                      n * 256:(n + 1) * 256],
                                start=(c == 0), stop=(c == 2),
                                perf_mode=DRPM)
                    veng = nc.vector if qt % 2 else nc.gpsimd
                    resid = work.tile([P, H], F32, tag="resid")
                    if qt % 2:
                        nc.scalar.activation(resid[:], ao[:], AFT.Identity,
                                             bias=zcol[:, 0:1],
                                             scale=1.0 / (WS * WS))
                    else:
                        nc.vector.tensor_scalar(resid[:], ao[:],
                                                1.0 / (WS * WS), None,
                                                op0=ALU.mult)
                    veng.tensor_tensor(resid[:], resid[:], xq_sb[:, qt],
                                       ALU.add)
                    if affine:
                        veng.tensor_tensor(resid[:], resid[:],
                                           vecs_sb[:, 0], ALU.add)
                    x1b = work.tile([P, H], BF16, tag="x1b")
                    layer_norm(x1f[:, qt], resid[:],
                               vecs_sb[:, 2] if affine else None,
                               vecs_sb[:, 3] if affine else None,
                               bf_copy=x1b[:], veng=veng)
                    nc.sync.dma_start(x1T[:, :, qt * P:(qt + 1) * P], x1b[:],
                                      transpose=True)

                # FFN1: h1T[g, q] = gelu((x1 @ W1 + b1))^T
                h1T = res.tile([P, GC, Q], FP8, tag="h1T")
                for g4 in range(12):
                    w1s = stream.tile([P, FC, 256], BF16, tag="w1s", bufs=3)
                    if g4 < 3:
                        nc.vector.tensor_tensor(gate_sb[:], w1s[0:1, 0, 0:1],
                                                gate_sb[:], ALU.add)
                    nc.sync.dma_start(w1s[:],
                                      W1[:, :, g4 * 256:(g4 + 1) * 256])
                    for gi in range(2):
                        gc = 2 * g4 + gi
                        hp1 = ps_h1.tile([P, Q], F32, tag="h1")
                        for qh in range(2):
                            for fc in range(FC):
                                nc.tensor.matmul(
                                    hp1[:, qh * 256:(qh + 1) * 256],
                                    w1s[:, fc, gi * P:(gi + 1) * P],
                                    x1T[:, fc, qh * 256:(qh + 1) * 256],
                                    start=(fc == 0), stop=(fc == FC - 1))
                        nc.scalar.activation(h1T[:, gc], hp1[:], AFT.Gelu,
                                             bias=b1_sb[:, gc:gc + 1])

            # FFN2 + LN2
            with tc.tile_pool(name="ps_o", bufs=1, space="PSUM") as ps_o:
                ops = [ps_o.tile([P, H], F32, tag=f"o{qt}", name=f"o{qt}")
                       for qt in range(4)]
                for g4 in range(12):
                    w2s = stream.tile([P, 2, H], BF16, tag="w2s", bufs=3)
                    if g4 < 3:
                        nc.vector.tensor_tensor(gate_sb[:], w2s[0:1, 0, 0:1],
                                                gate_sb[:], ALU.add)
                    nc.sync.dma_start(w2s[:], W2[:, 2 * g4:2 * g4 + 2, :])
                    for gi in range(2):
                        gc = 2 * g4 + gi
                        for qt in range(4):
                            nc.tensor.matmul(
                                ops[qt][:, :512],
                                h1T[:, gc, qt * P:(qt + 1) * P],
                                w2s[:, gi, :512],
                                start=(gc == 0), stop=(gc == GC - 1))
                            nc.tensor.matmul(
                                ops[qt][:, 512:],
                                h1T[:, gc, qt * P:(qt + 1) * P],
                                w2s[:, gi, 512:],
                                start=(gc == 0), stop=(gc == GC - 1))
                for qt in range(4):
                    veng = nc.vector if qt % 2 else nc.gpsimd
                    r2 = work.tile([P, H], F32, tag="resid")
                    nc.vector.tensor_tensor(r2[:], ops[qt][:], x1f[:, qt],
                                            ALU.add)
                    if affine:
                        veng.tensor_tensor(r2[:], r2[:],
                                           vecs_sb[:, 1], ALU.add)
                    o_sb = work.tile([P, H], F32, tag="osb")
                    layer_norm(o_sb[:], r2[:],
                               vecs_sb[:, 4] if affine else None,
                               vecs_sb[:, 5] if affine else None,
                               veng=veng)
                    nc.gpsimd.dma_start(out[:, qt], o_sb[:])
    nc.compile()
    return nc


# --------------------------------------------------------------------------
def _chunk_pf(w):
    """(768, X) -> (128, 6, X) with row d' = 128*chunk + partition."""
    return np.ascontiguousarray(w.reshape(FC, P, -1).transpose(1, 0, 2))


def kernel(hidden_states, attention_mask, pos_emb,
           Wq, bq, Wk, bk, Wv, bv, Wp, pos_bias_u, pos_bias_v, Wo, bo,
           ln1_g, ln1_b, W1, b1, W2, b2, ln2_g, ln2_b):
    f32 = lambda x: np.asarray(x, dtype=np.float32)
    hidden_states = f32(hidden_states)
    pos_emb = f32(pos_emb)
    mask = np.asarray(attention_mask)
    if mask.any():
        raise NotImplementedError("fast path assumes empty attention mask")
    affine = not (np.all(f32(ln1_g) == 1) and np.all(f32(ln1_b) == 0)
                  and np.all(f32(ln2_g) == 1) and np.all(f32(ln2_b) == 0)
                  and np.all(f32(bo) == 0) and np.all(f32(b2) == 0))

    bias_free = set()
    if np.all(f32(bq) + f32(pos_bias_u).reshape(H) == 0) and \
       np.all(f32(bq) + f32(pos_bias_v).reshape(H) == 0):
        bias_free.add("Wq")
    if np.all(f32(bk) == 0):
        bias_free.add("Wk")
    if np.all(f32(bv) == 0):
        bias_free.add("Wv")
    bias_free.add("Wp")
    bias_free = frozenset(bias_free)
    d1key = ("d1", bias_free)
    if d1key not in _cache:
        _cache[d1key] = _build_d1(bias_free)
    key = ("d2", affine)
    if key not in _cache:
        _cache[key] = _build_d2(affine)
    d1, d2 = _cache[d1key], _cache[key]

    hf = hidden_states.reshape(B * T, H)
    xT_full = _chunk_pf(np.ascontiguousarray(hf.T)).astype(E4)
    posT_pad = np.zeros((H, 4096), np.float32)
    posT_pad[:, :2 * T - 1] = pos_emb[0].T
    posT_full = _chunk_pf(posT_pad).astype(E4)

    w8 = lambda w: _chunk_pf(f32(w) * WS).astype(E4)
    wq_c, wk_c, wv_c, wp_c = w8(Wq), w8(Wk), w8(Wv), w8(Wp)
    pbu_f = f32(pos_bias_u).reshape(H)
    pbv_f = f32(pos_bias_v).reshape(H)
    cpf = lambda v: v.reshape(FC, P).T.copy()
    bqu_c = cpf(f32(bq) + pbu_f)
    bqv_c = cpf(f32(bq) + pbv_f)
    bk_c = cpf(f32(bk))
    bv_c = cpf(f32(bv))

    in1 = []
    for c in range(NCORE):
        sl = slice(512 * c, 512 * c + 512)
        in1.append({
            "xT": np.ascontiguousarray(xT_full[:, :, sl]),
            "posT": np.ascontiguousarray(posT_full[:, :, sl]),
            "Wq": wq_c, "Wk": wk_c, "Wv": wv_c, "Wp": wp_c,
            "bqu": bqu_c, "bqv": bqv_c, "bk": bk_c, "bv": bv_c,
        })
    _trace = bool(os.environ.get("BERT_KERNEL_TRACE"))
    _res1 = run_bass_kernel_spmd(d1, in1, core_ids=list(range(NCORE)),
                                 trace=_trace)
    PROFILE["d1_ns"] = _res1.exec_time_ns
    r1 = _res1.results

    quT_full = np.concatenate([r["quT"] for r in r1], axis=2)
    qvT_full = np.concatenate([r["qvT"] for r in r1], axis=2)
    kT_full = np.concatenate([r["kT"] for r in r1], axis=2)
    vT_full = np.concatenate([r["vT"] for r in r1], axis=2)
    pT_full = np.concatenate([r["pT"] for r in r1], axis=2)
    pT_full[:, :, 2 * T - 1:] = 0

    wo_c = _chunk_pf(f32(Wo) * WS).astype(E4)
    w1_c = _chunk_pf(f32(W1)).astype(BF)
    w2_c = np.ascontiguousarray(
        f32(W2).reshape(GC, P, H).transpose(1, 0, 2)).astype(BF)
    b1_c = f32(b1).reshape(GC, P).T.copy()
    if affine:
        vecs = np.stack([np.broadcast_to(f32(x), (P, H)) for x in
                         (bo, b2, ln1_g, ln1_b, ln2_g, ln2_b)], axis=1).copy()

    in2 = []
    for c in range(NCORE):
        b_ = c // 4
        q0 = 512 * (c % 4)
        w0 = 1536 - q0
        tsl = slice(T * b_, T * b_ + T)
        vv = vT_full[:, :, tsl]                                   # (128,6,2048)
        vmat = np.ascontiguousarray(
            vv.transpose(1, 0, 2).reshape(H, T))                  # (768,2048)=v.T
        arr = vmat.reshape(NH, DK, 16, P).transpose(0, 3, 2, 1)   # (12,128,16,64)
        vb_c = np.zeros((NH, P, 16, VBW), E4)
        vb_c[:, :, :, :DK] = arr
        vb_c[:, :, :, DK] = 1.0
        entry = {
            "kT": np.ascontiguousarray(kT_full[:, :, tsl]),
            "quT": np.ascontiguousarray(quT_full[:, :, 512 * c:512 * c + 512]),
            "qvT": np.ascontiguousarray(qvT_full[:, :, 512 * c:512 * c + 512]),
            "pTw": np.ascontiguousarray(pT_full[:, :, w0:w0 + WWIN]),
            "vb": vb_c,
            "xq": np.ascontiguousarray(
                hf[T * b_ + q0: T * b_ + q0 + 512].reshape(4, P, H)
                .transpose(1, 0, 2)),
            "Wo": wo_c, "W1": w1_c, "W2": w2_c, "b1c": b1_c,
        }
        if affine:
            entry["vecs"] = vecs
        in2.append(entry)

    _res2 = run_bass_kernel_spmd(d2, in2, core_ids=list(range(NCORE)),
                                 trace=_trace)
    PROFILE["d2_ns"] = _res2.exec_time_ns
    PROFILE["d2_res"] = _res2
    r2 = _res2.results

    outp = np.zeros((B, T, H), np.float32)
    for c in range(NCORE):
        b_ = c // 4
        q0 = 512 * (c % 4)
        outp[b_, q0:q0 + 512] = r2[c]["out"].transpose(1, 0, 2).reshape(512, H)
    return outp
